# revision 7
# baseline (speedup 1.0000x reference)
"""AnchorTransformer kernel for 8 TRN2 NeuronCores.

Data-parallel over the flattened pixel dim N = B*H*W = 32768 -> 4096/core.

Math (per pixel n with instance index i = max(lab-1, 0)):
    q = f W_q^T + b_q
    S[n, j] = scale * q . K_all[j]  for all J=512 anchor rows (64 inst x 8)
    masked softmax over the 8 columns of instance i, then attn @ V rows,
    out-proj, background zeroing, residual.

Folds (all computed on device, once per core):
    KW   = scale * (A W_k^T + b_k) W_q          (J, C)  score weights
    sb_j = scale * (A W_k^T + b_k)_j . b_q      (J,)    score bias (exp bias)
    V2   = (A W_v^T + b_v) W_o^T + 1 (x) b_o    (J, C)  out-proj folded into V
           (valid because attention weights sum to 1)
    mask = +30 on the 8 selected columns via one-hot(inst) matmul; softmax
           shift-invariance makes this equivalent to -inf masking, with
           e^-30 leakage ~ 1e-13.
    V2 gets an extra ones column so the attention matmul also produces the
    softmax denominator; normalize + gate + residual fuse into one DVE op.

Matmuls run in bf16 (f32 PSUM accumulation); the residual add and softmax
normalization stay f32.
"""

import numpy as np
import ml_dtypes
import concourse.bass as bass
import concourse.tile as tile
from concourse import bacc, mybir
from concourse.bass_utils import run_bass_kernel_spmd

NCORES = 8
N_FULL = 32768
NP = N_FULL // NCORES  # 4096 pixels per core
C = 256
M = 64
L = 8
J = M * L  # 512
TP = 256   # pixels per macro tile
NMT = NP // TP  # 16
F32 = mybir.dt.float32
BF16 = mybir.dt.bfloat16
SCALE = 1.0 / 16.0
BIG = 30.0

AF = mybir.ActivationFunctionType
OP = mybir.AluOpType


def build_nc():
    from contextlib import ExitStack

    nc = bacc.Bacc()
    fT = nc.declare_dram_parameter("fT", [C, NP], F32, isOutput=False)
    fpm = nc.declare_dram_parameter("fpm", [NP, C], F32, isOutput=False)
    ET = nc.declare_dram_parameter("ET", [M, NP], BF16, isOutput=False)
    gate = nc.declare_dram_parameter("gate", [NMT, 2, 128, 1], F32, isOutput=False)
    anchorsT = nc.declare_dram_parameter("anchorsT", [C, J], F32, isOutput=False)
    Wq = nc.declare_dram_parameter("Wq", [C, C], F32, isOutput=False)
    WkT = nc.declare_dram_parameter("WkT", [C, C], F32, isOutput=False)
    WvT = nc.declare_dram_parameter("WvT", [C, C], F32, isOutput=False)
    bq = nc.declare_dram_parameter("bq", [C, 1], F32, isOutput=False)
    bk = nc.declare_dram_parameter("bk", [C, 1], F32, isOutput=False)
    bv = nc.declare_dram_parameter("bv", [C, 1], F32, isOutput=False)
    OWaug = nc.declare_dram_parameter("OWaug", [C, C + 1], F32, isOutput=False)
    obaug = nc.declare_dram_parameter("obaug", [1, C + 1], F32, isOutput=False)
    R30 = nc.declare_dram_parameter("R30", [M, J], BF16, isOutput=False)
    ones1 = nc.declare_dram_parameter("ones1", [1, 128], BF16, isOutput=False)
    out = nc.declare_dram_parameter("out", [NP, C], F32, isOutput=True)

    with tile.TileContext(nc) as tc, ExitStack() as es:
        cp = es.enter_context(tc.tile_pool(name="const", bufs=1))

        # ---- per-core precompute of folded tables ----
        with tc.tile_pool(name="pre", bufs=1) as pp, \
             tc.tile_pool(name="pre_ps", space="PSUM", bufs=2) as pps:

            # load an f32 (256, w) param and convert to 2 bf16 (128, w) tiles
            _engs = [
                lambda o, i: nc.vector.tensor_copy(o, i),
                lambda o, i: nc.scalar.copy(o, i),
                lambda o, i: nc.gpsimd.tensor_copy(o, i),
            ]
            _ec = [0]

            def load2bf(dram, w, tagp):
                ts = []
                for i in range(2):
                    tf = pp.tile([128, w], F32, tag="ldf32", bufs=4)
                    nc.sync.dma_start(tf[:], dram[i * 128:(i + 1) * 128, :])
                    t = pp.tile([128, w], BF16, tag=f"{tagp}{i}")
                    _engs[_ec[0] % 3](t[:], tf[:])
                    _ec[0] += 1
                    ts.append(t)
                return ts

            anchT_sb = load2bf(anchorsT, J, "anchT")
            WkT_sb = load2bf(WkT, C, "wkt")
            WvT_sb = load2bf(WvT, C, "wvt")
            Wq_sb = load2bf(Wq, C, "wq")
            OW_sb = load2bf(OWaug, C + 1, "ow")
            bq_sb = load2bf(bq, 1, "bq")

            bk_sb, bv_sb = [], []
            for i in range(2):
                t = pp.tile([128, 1], F32, tag=f"bk{i}")
                nc.sync.dma_start(t[:], bk[i * 128:(i + 1) * 128, :])
                bk_sb.append(t)
                t = pp.tile([128, 1], F32, tag=f"bv{i}")
                nc.sync.dma_start(t[:], bv[i * 128:(i + 1) * 128, :])
                bv_sb.append(t)

            obf = pp.tile([1, C + 1], F32, tag="obf")
            nc.sync.dma_start(obf[:], obaug[:, :])
            ob_sb = pp.tile([1, C + 1], BF16, tag="ob")
            nc.vector.tensor_copy(ob_sb[:], obf[:])

            R30_sb = cp.tile([M, J], BF16, tag="r30")
            nc.sync.dma_start(R30_sb[:], R30[:, :])
            ones1_sb = pp.tile([1, 128], BF16, tag="ones1")
            nc.sync.dma_start(ones1_sb[:], ones1[:, :])

            # K_allT / V_allT: (c', j) channel-major projected anchors
            K_allT_sb, V_allT_sb = [], []
            for wT_sb, b_sb, dst, tagp in (
                (WkT_sb, bk_sb, K_allT_sb, "kallt"),
                (WvT_sb, bv_sb, V_allT_sb, "vallt"),
            ):
                for ct in range(2):
                    ps = pps.tile([128, J], F32, tag="preps", bufs=2)
                    for et in range(2):
                        nc.tensor.matmul(
                            ps[:],
                            wT_sb[et][:, ct * 128:(ct + 1) * 128],
                            anchT_sb[et][:],
                            start=(et == 0), stop=(et == 1),
                        )
                    t = pp.tile([128, J], BF16, tag=f"{tagp}{ct}")
                    nc.vector.tensor_scalar_add(t[:], ps[:], b_sb[ct][:, 0:1])
                    dst.append(t)

            # KWT: (e, j) = scale * Wq^T K_allT
            KWT_sb = []
            for et in range(2):
                ps = pps.tile([128, J], F32, tag="preps", bufs=2)
                for ct in range(2):
                    nc.tensor.matmul(
                        ps[:],
                        Wq_sb[ct][:, et * 128:(et + 1) * 128],
                        K_allT_sb[ct][:],
                        start=(ct == 0), stop=(ct == 1),
                    )
                t = cp.tile([128, J], BF16, tag=f"kwt{et}")
                nc.vector.tensor_scalar_mul(t[:], ps[:], SCALE)
                KWT_sb.append(t)

            # sb_jm: per-j score bias, j-major (4 x (128, 1)) f32
            sb_jm = []
            for jt in range(4):
                ps1 = pps.tile([128, 1], F32, tag="preps1", bufs=2)
                for ct in range(2):
                    nc.tensor.matmul(
                        ps1[:],
                        K_allT_sb[ct][:, jt * 128:(jt + 1) * 128],
                        bq_sb[ct][:],
                        start=(ct == 0), stop=(ct == 1),
                    )
                t = cp.tile([128, 1], F32, tag=f"sbjm{jt}")
                nc.vector.tensor_scalar_mul(t[:], ps1[:], SCALE)
                sb_jm.append(t)

            # V2_aug: (j, 257) value rows with out-proj + out_b folded,
            # last column = 1 (softmax denominator accumulator)
            V2_sb = []
            for jt in range(4):
                ps = pps.tile([128, C + 1], F32, tag="preps257", bufs=2)
                for ct in range(2):
                    nc.tensor.matmul(
                        ps[:],
                        V_allT_sb[ct][:, jt * 128:(jt + 1) * 128],
                        OW_sb[ct][:],
                        start=(ct == 0), stop=False,
                    )
                nc.tensor.matmul(
                    ps[:], ones1_sb[:], ob_sb[:],
                    start=False, stop=True,
                )
                t = cp.tile([128, C + 1], BF16, tag=f"v2_{jt}")
                nc.vector.tensor_copy(t[:], ps[:])
                V2_sb.append(t)

        # ---- main per-pixel loop ----
        io = es.enter_context(tc.tile_pool(name="io", bufs=3))
        sps = es.enter_context(tc.tile_pool(name="sps", space="PSUM", bufs=4))
        ops = es.enter_context(tc.tile_pool(name="ops", space="PSUM", bufs=2))

        for mt in range(NMT):
            fT_t = []
            for et in range(2):
                tf = io.tile([128, TP], F32, tag=f"ftf{et}", bufs=3)
                nc.sync.dma_start(
                    tf[:], fT[et * 128:(et + 1) * 128, mt * TP:(mt + 1) * TP])
                t = io.tile([128, TP], BF16, tag=f"ft{et}", bufs=3)
                nc.vector.tensor_copy(t[:], tf[:])
                fT_t.append(t)
            ET_t = io.tile([M, TP], BF16, tag="et", bufs=3)
            nc.sync.dma_start(ET_t[:], ET[:, mt * TP:(mt + 1) * TP])
            gate_t = io.tile([128, 2], F32, tag="gate", bufs=3)
            for st in range(2):
                nc.sync.dma_start(gate_t[:, st:st + 1], gate[mt, st, :, :])

            P_t = []
            for jt in range(4):
                sp = sps.tile([128, TP], F32, tag="s", bufs=4)
                nc.tensor.matmul(
                    sp[:], KWT_sb[0][:, jt * 128:(jt + 1) * 128],
                    fT_t[0][:], start=True, stop=False)
                nc.tensor.matmul(
                    sp[:], KWT_sb[1][:, jt * 128:(jt + 1) * 128],
                    fT_t[1][:], start=False, stop=False)
                nc.tensor.matmul(
                    sp[:], R30_sb[:, jt * 128:(jt + 1) * 128],
                    ET_t[:], start=False, stop=True)
                pt = io.tile([128, TP], BF16, tag=f"p{jt}", bufs=2)
                nc.scalar.activation(pt[:], sp[:], AF.Exp, bias=sb_jm[jt][:, 0:1])
                P_t.append(pt)

            for st in range(2):
                op = ops.tile([128, C + 1], F32, tag="o", bufs=2)
                for jt in range(4):
                    nc.tensor.matmul(
                        op[:], P_t[jt][:, st * 128:(st + 1) * 128],
                        V2_sb[jt][:], start=(jt == 0), stop=(jt == 3))
                fpm_t = io.tile([128, C], F32, tag="fpm", bufs=3)
                r0 = mt * TP + st * 128
                nc.sync.dma_start(fpm_t[:], fpm[r0:r0 + 128, :])
                recip = io.tile([128, 1], F32, tag="recip", bufs=3)
                nc.vector.reciprocal(recip[:], op[:, C:C + 1])
                rg = io.tile([128, 1], F32, tag="rg", bufs=3)
                nc.vector.tensor_mul(rg[:], recip[:], gate_t[:, st:st + 1])
                ot = io.tile([128, C], F32, tag="ot", bufs=3)
                nc.vector.scalar_tensor_tensor(
                    ot[:], op[:, 0:C], rg[:, 0:1], fpm_t[:], OP.mult, OP.add)
                nc.sync.dma_start(out[r0:r0 + 128, :], ot[:])

    nc.compile()
    return nc


_CACHE = {}


def _build():
    if "nc" not in _CACHE:
        _CACHE["nc"] = build_nc()
    return _CACHE["nc"]


def _prep_maps(anchors, features, instances_in_view, in_proj_w, in_proj_b,
               out_w, out_b):
    f32 = np.float32
    bf16 = ml_dtypes.bfloat16
    anchors = np.asarray(anchors, f32)
    features = np.asarray(features, f32)
    iiv = np.asarray(instances_in_view, np.int32)
    in_proj_w = np.asarray(in_proj_w, f32)
    in_proj_b = np.asarray(in_proj_b, f32)
    out_w = np.asarray(out_w, f32)
    out_b = np.asarray(out_b, f32)

    f_flat = features.reshape(N_FULL, C)
    fT_full = np.ascontiguousarray(f_flat.T)
    lab = iiv.reshape(-1)
    idx = np.maximum(lab - 1, 0)
    ET_full = (idx[None, :] == np.arange(M, dtype=np.int32)[:, None]).astype(bf16)
    gate_full = (lab > 0).astype(f32)

    anchorsT = np.ascontiguousarray(anchors.reshape(J, C).T)
    Wq_h = np.ascontiguousarray(in_proj_w[:C])
    WkT_h = np.ascontiguousarray(in_proj_w[C:2 * C].T)
    WvT_h = np.ascontiguousarray(in_proj_w[2 * C:].T)
    bq_h = np.ascontiguousarray(in_proj_b[:C].reshape(C, 1))
    bk_h = np.ascontiguousarray(in_proj_b[C:2 * C].reshape(C, 1))
    bv_h = np.ascontiguousarray(in_proj_b[2 * C:].reshape(C, 1))
    OWaug_h = np.concatenate([out_w.T, np.zeros((C, 1), f32)], axis=1)
    obaug_h = np.concatenate([out_b, np.ones(1, f32)]).reshape(1, C + 1)
    R30_h = (np.repeat(np.eye(M, dtype=f32), L, axis=1) * f32(BIG)).astype(bf16)
    ones1_h = np.ones((1, 128), bf16)

    in_maps = []
    for i in range(NCORES):
        sl = slice(i * NP, (i + 1) * NP)
        in_maps.append({
            "fT": np.ascontiguousarray(fT_full[:, sl]),
            "fpm": np.ascontiguousarray(f_flat[sl]),
            "ET": np.ascontiguousarray(ET_full[:, sl]),
            "gate": np.ascontiguousarray(
                gate_full[sl].reshape(NMT, 2, 128, 1)),
            "anchorsT": anchorsT, "Wq": Wq_h, "WkT": WkT_h, "WvT": WvT_h,
            "bq": bq_h, "bk": bk_h, "bv": bv_h,
            "OWaug": OWaug_h, "obaug": obaug_h,
            "R30": R30_h, "ones1": ones1_h,
        })
    return in_maps, features.shape


def _run(in_maps, **kw):
    nc = _build()
    return run_bass_kernel_spmd(nc, in_maps, core_ids=list(range(NCORES)), **kw)


def kernel(**inputs):
    in_maps, shp = _prep_maps(**inputs)
    res = _run(in_maps)
    outs = [np.asarray(r["out"]) for r in res.results]
    return np.concatenate(outs, axis=0).reshape(shp).astype(np.float32)


# revision 8
# speedup vs baseline: 1.0556x; 1.0556x over previous
"""AnchorTransformer kernel for 8 TRN2 NeuronCores.

Data-parallel over the flattened pixel dim N = B*H*W = 32768 -> 4096/core.

Math (per pixel n with instance index i = max(lab-1, 0)):
    q = f W_q^T + b_q
    S[n, j] = scale * q . K_all[j]  for all J=512 anchor rows (64 inst x 8)
    masked softmax over the 8 columns of instance i, then attn @ V rows,
    out-proj, background zeroing, residual.

Folds (all computed on device, once per core):
    KW   = scale * (A W_k^T + b_k) W_q          (J, C)  score weights
    sb_j = scale * (A W_k^T + b_k)_j . b_q      (J,)    score bias (exp bias)
    V2   = (A W_v^T + b_v) W_o^T + 1 (x) b_o    (J, C)  out-proj folded into V
           (valid because attention weights sum to 1)
    mask = +30 on the 8 selected columns via one-hot(inst) matmul; softmax
           shift-invariance makes this equivalent to -inf masking, with
           e^-30 leakage ~ 1e-13.
    V2 gets an extra ones column so the attention matmul also produces the
    softmax denominator; normalize + gate + residual fuse into one DVE op.

Matmuls run in bf16 (f32 PSUM accumulation); the residual add and softmax
normalization stay f32.
"""

import numpy as np
import ml_dtypes
import concourse.bass as bass
import concourse.tile as tile
from concourse import bacc, mybir
from concourse.bass_utils import run_bass_kernel_spmd

NCORES = 8
N_FULL = 32768
NP = N_FULL // NCORES  # 4096 pixels per core
C = 256
M = 64
L = 8
J = M * L  # 512
TP = 512   # pixels per macro tile
NMT = NP // TP  # 16
F32 = mybir.dt.float32
BF16 = mybir.dt.bfloat16
SCALE = 1.0 / 16.0
BIG = 30.0

AF = mybir.ActivationFunctionType
OP = mybir.AluOpType


def build_nc():
    from contextlib import ExitStack

    nc = bacc.Bacc()
    fT = nc.declare_dram_parameter("fT", [C, NP], F32, isOutput=False)
    fpm = nc.declare_dram_parameter("fpm", [NP, C], F32, isOutput=False)
    ET = nc.declare_dram_parameter("ET", [M, NP], BF16, isOutput=False)
    gate = nc.declare_dram_parameter("gate", [NMT, 4, 128, 1], F32, isOutput=False)
    anchorsT = nc.declare_dram_parameter("anchorsT", [C, J], F32, isOutput=False)
    Wq = nc.declare_dram_parameter("Wq", [C, C], F32, isOutput=False)
    WkT = nc.declare_dram_parameter("WkT", [C, C], F32, isOutput=False)
    WvT = nc.declare_dram_parameter("WvT", [C, C], F32, isOutput=False)
    bq = nc.declare_dram_parameter("bq", [C, 1], F32, isOutput=False)
    bk = nc.declare_dram_parameter("bk", [C, 1], F32, isOutput=False)
    bv = nc.declare_dram_parameter("bv", [C, 1], F32, isOutput=False)
    OWaug = nc.declare_dram_parameter("OWaug", [C, C + 1], F32, isOutput=False)
    obaug = nc.declare_dram_parameter("obaug", [1, C + 1], F32, isOutput=False)
    R30 = nc.declare_dram_parameter("R30", [M, J], BF16, isOutput=False)
    ones1 = nc.declare_dram_parameter("ones1", [1, 128], BF16, isOutput=False)
    out = nc.declare_dram_parameter("out", [NP, C], F32, isOutput=True)

    with tile.TileContext(nc) as tc, ExitStack() as es:
        cp = es.enter_context(tc.tile_pool(name="const", bufs=1))

        # ---- per-core precompute of folded tables ----
        with tc.tile_pool(name="pre", bufs=1) as pp, \
             tc.tile_pool(name="pre_ps", space="PSUM", bufs=2) as pps:

            # load an f32 (256, w) param and convert to 2 bf16 (128, w) tiles
            _engs = [
                lambda o, i: nc.vector.tensor_copy(o, i),
                lambda o, i: nc.scalar.copy(o, i),
                lambda o, i: nc.gpsimd.tensor_copy(o, i),
            ]
            _ec = [0]

            def load2bf(dram, w, tagp):
                ts = []
                for i in range(2):
                    tf = pp.tile([128, w], F32, tag="ldf32", bufs=4)
                    nc.sync.dma_start(tf[:], dram[i * 128:(i + 1) * 128, :])
                    t = pp.tile([128, w], BF16, tag=f"{tagp}{i}")
                    _engs[_ec[0] % 3](t[:], tf[:])
                    _ec[0] += 1
                    ts.append(t)
                return ts

            anchT_sb = load2bf(anchorsT, J, "anchT")
            WkT_sb = load2bf(WkT, C, "wkt")
            WvT_sb = load2bf(WvT, C, "wvt")
            Wq_sb = load2bf(Wq, C, "wq")
            OW_sb = load2bf(OWaug, C + 1, "ow")
            bq_sb = load2bf(bq, 1, "bq")

            bk_sb, bv_sb = [], []
            for i in range(2):
                t = pp.tile([128, 1], F32, tag=f"bk{i}")
                nc.sync.dma_start(t[:], bk[i * 128:(i + 1) * 128, :])
                bk_sb.append(t)
                t = pp.tile([128, 1], F32, tag=f"bv{i}")
                nc.sync.dma_start(t[:], bv[i * 128:(i + 1) * 128, :])
                bv_sb.append(t)

            obf = pp.tile([1, C + 1], F32, tag="obf")
            nc.sync.dma_start(obf[:], obaug[:, :])
            ob_sb = pp.tile([1, C + 1], BF16, tag="ob")
            nc.vector.tensor_copy(ob_sb[:], obf[:])

            R30_sb = cp.tile([M, J], BF16, tag="r30")
            nc.sync.dma_start(R30_sb[:], R30[:, :])
            ones1_sb = pp.tile([1, 128], BF16, tag="ones1")
            nc.sync.dma_start(ones1_sb[:], ones1[:, :])

            # K_allT / V_allT: (c', j) channel-major projected anchors
            K_allT_sb, V_allT_sb = [], []
            for wT_sb, b_sb, dst, tagp in (
                (WkT_sb, bk_sb, K_allT_sb, "kallt"),
                (WvT_sb, bv_sb, V_allT_sb, "vallt"),
            ):
                for ct in range(2):
                    ps = pps.tile([128, J], F32, tag="preps", bufs=2)
                    for et in range(2):
                        nc.tensor.matmul(
                            ps[:],
                            wT_sb[et][:, ct * 128:(ct + 1) * 128],
                            anchT_sb[et][:],
                            start=(et == 0), stop=(et == 1),
                        )
                    t = pp.tile([128, J], BF16, tag=f"{tagp}{ct}")
                    nc.vector.tensor_scalar_add(t[:], ps[:], b_sb[ct][:, 0:1])
                    dst.append(t)

            # KWT: (e, j) = scale * Wq^T K_allT
            KWT_sb = []
            for et in range(2):
                ps = pps.tile([128, J], F32, tag="preps", bufs=2)
                for ct in range(2):
                    nc.tensor.matmul(
                        ps[:],
                        Wq_sb[ct][:, et * 128:(et + 1) * 128],
                        K_allT_sb[ct][:],
                        start=(ct == 0), stop=(ct == 1),
                    )
                t = cp.tile([128, J], BF16, tag=f"kwt{et}")
                nc.vector.tensor_scalar_mul(t[:], ps[:], SCALE)
                KWT_sb.append(t)

            # sb_jm: per-j score bias, j-major (4 x (128, 1)) f32
            sb_jm = []
            for jt in range(4):
                ps1 = pps.tile([128, 1], F32, tag="preps1", bufs=2)
                for ct in range(2):
                    nc.tensor.matmul(
                        ps1[:],
                        K_allT_sb[ct][:, jt * 128:(jt + 1) * 128],
                        bq_sb[ct][:],
                        start=(ct == 0), stop=(ct == 1),
                    )
                t = cp.tile([128, 1], F32, tag=f"sbjm{jt}")
                nc.vector.tensor_scalar_mul(t[:], ps1[:], SCALE)
                sb_jm.append(t)

            # V2_aug: (j, 257) value rows with out-proj + out_b folded,
            # last column = 1 (softmax denominator accumulator)
            V2_sb = []
            for jt in range(4):
                ps = pps.tile([128, C + 1], F32, tag="preps257", bufs=2)
                for ct in range(2):
                    nc.tensor.matmul(
                        ps[:],
                        V_allT_sb[ct][:, jt * 128:(jt + 1) * 128],
                        OW_sb[ct][:],
                        start=(ct == 0), stop=False,
                    )
                nc.tensor.matmul(
                    ps[:], ones1_sb[:], ob_sb[:],
                    start=False, stop=True,
                )
                t = cp.tile([128, C + 1], BF16, tag=f"v2_{jt}")
                nc.vector.tensor_copy(t[:], ps[:])
                V2_sb.append(t)

        # ---- main per-pixel loop ----
        io = es.enter_context(tc.tile_pool(name="io", bufs=4))
        sps = es.enter_context(tc.tile_pool(name="sps", space="PSUM", bufs=5))
        ops = es.enter_context(tc.tile_pool(name="ops", space="PSUM", bufs=2))

        for mt in range(NMT):
            fT_t = []
            for et in range(2):
                tf = io.tile([128, TP], F32, tag=f"ftf{et}", bufs=3)
                nc.sync.dma_start(
                    tf[:], fT[et * 128:(et + 1) * 128, mt * TP:(mt + 1) * TP])
                t = io.tile([128, TP], BF16, tag=f"ft{et}", bufs=3)
                nc.gpsimd.tensor_copy(t[:], tf[:])
                fT_t.append(t)
            ET_t = io.tile([M, TP], BF16, tag="et", bufs=3)
            nc.sync.dma_start(ET_t[:], ET[:, mt * TP:(mt + 1) * TP])
            gate_t = io.tile([128, 4], F32, tag="gate", bufs=3)
            for st in range(4):
                nc.sync.dma_start(gate_t[:, st:st + 1], gate[mt, st, :, :])

            P_t = []
            for jt in range(4):
                sp = sps.tile([128, TP], F32, tag="s", bufs=5)
                nc.tensor.matmul(
                    sp[:], KWT_sb[0][:, jt * 128:(jt + 1) * 128],
                    fT_t[0][:], start=True, stop=False)
                nc.tensor.matmul(
                    sp[:], KWT_sb[1][:, jt * 128:(jt + 1) * 128],
                    fT_t[1][:], start=False, stop=False)
                nc.tensor.matmul(
                    sp[:], R30_sb[:, jt * 128:(jt + 1) * 128],
                    ET_t[:], start=False, stop=True)
                pt = io.tile([128, TP], BF16, tag=f"p{jt}", bufs=2)
                nc.scalar.activation(pt[:], sp[:], AF.Exp, bias=sb_jm[jt][:, 0:1])
                P_t.append(pt)

            for st in range(4):
                op = ops.tile([128, C + 1], F32, tag="o", bufs=2)
                for jt in range(4):
                    nc.tensor.matmul(
                        op[:], P_t[jt][:, st * 128:(st + 1) * 128],
                        V2_sb[jt][:], start=(jt == 0), stop=(jt == 3))
                fpm_t = io.tile([128, C], F32, tag="fpm", bufs=4)
                r0 = mt * TP + st * 128
                nc.sync.dma_start(fpm_t[:], fpm[r0:r0 + 128, :])
                recip = io.tile([128, 1], F32, tag="recip", bufs=4)
                nc.vector.reciprocal(recip[:], op[:, C:C + 1])
                rg = io.tile([128, 1], F32, tag="rg", bufs=4)
                nc.vector.tensor_mul(rg[:], recip[:], gate_t[:, st:st + 1])
                ot = io.tile([128, C], F32, tag="ot", bufs=4)
                nc.vector.scalar_tensor_tensor(
                    ot[:], op[:, 0:C], rg[:, 0:1], fpm_t[:], OP.mult, OP.add)
                nc.sync.dma_start(out[r0:r0 + 128, :], ot[:])

    nc.compile()
    return nc


_CACHE = {}


def _build():
    if "nc" not in _CACHE:
        _CACHE["nc"] = build_nc()
    return _CACHE["nc"]


def _prep_maps(anchors, features, instances_in_view, in_proj_w, in_proj_b,
               out_w, out_b):
    f32 = np.float32
    bf16 = ml_dtypes.bfloat16
    anchors = np.asarray(anchors, f32)
    features = np.asarray(features, f32)
    iiv = np.asarray(instances_in_view, np.int32)
    in_proj_w = np.asarray(in_proj_w, f32)
    in_proj_b = np.asarray(in_proj_b, f32)
    out_w = np.asarray(out_w, f32)
    out_b = np.asarray(out_b, f32)

    f_flat = features.reshape(N_FULL, C)
    fT_full = np.ascontiguousarray(f_flat.T)
    lab = iiv.reshape(-1)
    idx = np.maximum(lab - 1, 0)
    ET_full = (idx[None, :] == np.arange(M, dtype=np.int32)[:, None]).astype(bf16)
    gate_full = (lab > 0).astype(f32)

    anchorsT = np.ascontiguousarray(anchors.reshape(J, C).T)
    Wq_h = np.ascontiguousarray(in_proj_w[:C])
    WkT_h = np.ascontiguousarray(in_proj_w[C:2 * C].T)
    WvT_h = np.ascontiguousarray(in_proj_w[2 * C:].T)
    bq_h = np.ascontiguousarray(in_proj_b[:C].reshape(C, 1))
    bk_h = np.ascontiguousarray(in_proj_b[C:2 * C].reshape(C, 1))
    bv_h = np.ascontiguousarray(in_proj_b[2 * C:].reshape(C, 1))
    OWaug_h = np.concatenate([out_w.T, np.zeros((C, 1), f32)], axis=1)
    obaug_h = np.concatenate([out_b, np.ones(1, f32)]).reshape(1, C + 1)
    R30_h = (np.repeat(np.eye(M, dtype=f32), L, axis=1) * f32(BIG)).astype(bf16)
    ones1_h = np.ones((1, 128), bf16)

    in_maps = []
    for i in range(NCORES):
        sl = slice(i * NP, (i + 1) * NP)
        in_maps.append({
            "fT": np.ascontiguousarray(fT_full[:, sl]),
            "fpm": np.ascontiguousarray(f_flat[sl]),
            "ET": np.ascontiguousarray(ET_full[:, sl]),
            "gate": np.ascontiguousarray(
                gate_full[sl].reshape(NMT, 4, 128, 1)),
            "anchorsT": anchorsT, "Wq": Wq_h, "WkT": WkT_h, "WvT": WvT_h,
            "bq": bq_h, "bk": bk_h, "bv": bv_h,
            "OWaug": OWaug_h, "obaug": obaug_h,
            "R30": R30_h, "ones1": ones1_h,
        })
    return in_maps, features.shape


def _run(in_maps, **kw):
    nc = _build()
    return run_bass_kernel_spmd(nc, in_maps, core_ids=list(range(NCORES)), **kw)


def kernel(**inputs):
    in_maps, shp = _prep_maps(**inputs)
    res = _run(in_maps)
    outs = [np.asarray(r["out"]) for r in res.results]
    return np.concatenate(outs, axis=0).reshape(shp).astype(np.float32)


# revision 10
# speedup vs baseline: 1.6361x; 1.5499x over previous
"""AnchorTransformer kernel for 8 TRN2 NeuronCores.

Data-parallel over the flattened pixel dim N = B*H*W = 32768 -> 4096/core.

Math (per pixel n with instance index i = max(lab-1, 0)):
    q = f W_q^T + b_q
    S[n, j] = scale * q . K_all[j]  for all J=512 anchor rows (64 inst x 8)
    masked softmax over the 8 columns of instance i, then attn @ V rows,
    out-proj, background zeroing, residual.

Folds (all computed on device, once per core):
    KW   = scale * (A W_k^T + b_k) W_q          (J, C)  score weights
    sb_j = scale * (A W_k^T + b_k)_j . b_q      (J,)    score bias (exp bias)
    V2   = (A W_v^T + b_v) W_o^T + 1 (x) b_o    (J, C)  out-proj folded into V
           (valid because attention weights sum to 1)
    mask = +30 on the 8 selected columns via one-hot(inst) matmul; softmax
           shift-invariance makes this equivalent to -inf masking, with
           e^-30 leakage ~ 1e-13.
    V2 gets an extra ones column so the attention matmul also produces the
    softmax denominator; normalize + gate + residual fuse into one DVE op.

Matmuls run in bf16 (f32 PSUM accumulation); the residual add and softmax
normalization stay f32.
"""

import numpy as np
import ml_dtypes
import concourse.bass as bass
import concourse.tile as tile
from concourse import bacc, mybir
from concourse.bass_utils import run_bass_kernel_spmd

NCORES = 8
N_FULL = 32768
NP = N_FULL // NCORES  # 4096 pixels per core
C = 256
M = 64
L = 8
J = M * L  # 512
TP = 512   # pixels per macro tile
NMT = NP // TP  # 16
F32 = mybir.dt.float32
BF16 = mybir.dt.bfloat16
SCALE = 1.0 / 16.0
BIG = 30.0

AF = mybir.ActivationFunctionType
OP = mybir.AluOpType


def build_nc():
    from contextlib import ExitStack

    nc = bacc.Bacc()
    fT = nc.declare_dram_parameter("fT", [C, NP], BF16, isOutput=False)
    fpm = nc.declare_dram_parameter("fpm", [NMT, 128, 4 * C], F32, isOutput=False)
    ET = nc.declare_dram_parameter("ET", [M, NP], BF16, isOutput=False)
    gate = nc.declare_dram_parameter("gate", [NMT, 128, 4], F32, isOutput=False)
    anchorsT = nc.declare_dram_parameter("anchorsT", [C, J], F32, isOutput=False)
    Wq = nc.declare_dram_parameter("Wq", [C, C], F32, isOutput=False)
    WkT = nc.declare_dram_parameter("WkT", [C, C], F32, isOutput=False)
    WvT = nc.declare_dram_parameter("WvT", [C, C], F32, isOutput=False)
    bq = nc.declare_dram_parameter("bq", [C, 1], F32, isOutput=False)
    bk = nc.declare_dram_parameter("bk", [C, 1], F32, isOutput=False)
    bv = nc.declare_dram_parameter("bv", [C, 1], F32, isOutput=False)
    OWaug = nc.declare_dram_parameter("OWaug", [C, C + 1], F32, isOutput=False)
    obaug = nc.declare_dram_parameter("obaug", [1, C + 1], F32, isOutput=False)
    R30 = nc.declare_dram_parameter("R30", [M, J], BF16, isOutput=False)
    ones1 = nc.declare_dram_parameter("ones1", [1, 128], BF16, isOutput=False)
    out = nc.declare_dram_parameter("out", [NMT, 128, 4 * C], F32, isOutput=True)

    with tile.TileContext(nc) as tc, ExitStack() as es:
        cp = es.enter_context(tc.tile_pool(name="const", bufs=1))

        # ---- per-core precompute of folded tables ----
        with tc.tile_pool(name="pre", bufs=1) as pp, \
             tc.tile_pool(name="pre_ps", space="PSUM", bufs=2) as pps:

            # load an f32 (256, w) param and convert to 2 bf16 (128, w) tiles
            _engs = [
                lambda o, i: nc.vector.tensor_copy(o, i),
                lambda o, i: nc.scalar.copy(o, i),
                lambda o, i: nc.gpsimd.tensor_copy(o, i),
            ]
            _ec = [0]

            def load2bf(dram, w, tagp):
                ts = []
                for i in range(2):
                    tf = pp.tile([128, w], F32, tag="ldf32", bufs=4)
                    nc.sync.dma_start(tf[:], dram[i * 128:(i + 1) * 128, :])
                    t = pp.tile([128, w], BF16, tag=f"{tagp}{i}")
                    _engs[_ec[0] % 3](t[:], tf[:])
                    _ec[0] += 1
                    ts.append(t)
                return ts

            anchT_sb = load2bf(anchorsT, J, "anchT")
            WkT_sb = load2bf(WkT, C, "wkt")
            WvT_sb = load2bf(WvT, C, "wvt")
            Wq_sb = load2bf(Wq, C, "wq")
            OW_sb = load2bf(OWaug, C + 1, "ow")
            bq_sb = load2bf(bq, 1, "bq")

            bk_sb, bv_sb = [], []
            for i in range(2):
                t = pp.tile([128, 1], F32, tag=f"bk{i}")
                nc.sync.dma_start(t[:], bk[i * 128:(i + 1) * 128, :])
                bk_sb.append(t)
                t = pp.tile([128, 1], F32, tag=f"bv{i}")
                nc.sync.dma_start(t[:], bv[i * 128:(i + 1) * 128, :])
                bv_sb.append(t)

            obf = pp.tile([1, C + 1], F32, tag="obf")
            nc.sync.dma_start(obf[:], obaug[:, :])
            ob_sb = pp.tile([1, C + 1], BF16, tag="ob")
            nc.vector.tensor_copy(ob_sb[:], obf[:])

            R30_sb = cp.tile([M, J], BF16, tag="r30")
            nc.sync.dma_start(R30_sb[:], R30[:, :])
            ones1_sb = pp.tile([1, 128], BF16, tag="ones1")
            nc.sync.dma_start(ones1_sb[:], ones1[:, :])

            # K_allT / V_allT: (c', j) channel-major projected anchors
            K_allT_sb, V_allT_sb = [], []
            for wT_sb, b_sb, dst, tagp in (
                (WkT_sb, bk_sb, K_allT_sb, "kallt"),
                (WvT_sb, bv_sb, V_allT_sb, "vallt"),
            ):
                for ct in range(2):
                    ps = pps.tile([128, J], F32, tag="preps", bufs=2)
                    for et in range(2):
                        nc.tensor.matmul(
                            ps[:],
                            wT_sb[et][:, ct * 128:(ct + 1) * 128],
                            anchT_sb[et][:],
                            start=(et == 0), stop=(et == 1),
                        )
                    t = pp.tile([128, J], BF16, tag=f"{tagp}{ct}")
                    nc.vector.tensor_scalar_add(t[:], ps[:], b_sb[ct][:, 0:1])
                    dst.append(t)

            # KWT: (e, j) = scale * Wq^T K_allT
            KWT_sb = []
            for et in range(2):
                ps = pps.tile([128, J], F32, tag="preps", bufs=2)
                for ct in range(2):
                    nc.tensor.matmul(
                        ps[:],
                        Wq_sb[ct][:, et * 128:(et + 1) * 128],
                        K_allT_sb[ct][:],
                        start=(ct == 0), stop=(ct == 1),
                    )
                t = cp.tile([128, J], BF16, tag=f"kwt{et}")
                nc.vector.tensor_scalar_mul(t[:], ps[:], SCALE)
                KWT_sb.append(t)

            # sb_jm: per-j score bias, j-major (4 x (128, 1)) f32
            sb_jm = []
            for jt in range(4):
                ps1 = pps.tile([128, 1], F32, tag="preps1", bufs=2)
                for ct in range(2):
                    nc.tensor.matmul(
                        ps1[:],
                        K_allT_sb[ct][:, jt * 128:(jt + 1) * 128],
                        bq_sb[ct][:],
                        start=(ct == 0), stop=(ct == 1),
                    )
                t = cp.tile([128, 1], F32, tag=f"sbjm{jt}")
                nc.vector.tensor_scalar_mul(t[:], ps1[:], SCALE)
                sb_jm.append(t)

            # V2_aug: (j, 257) value rows with out-proj + out_b folded,
            # last column = 1 (softmax denominator accumulator)
            V2_sb = []
            for jt in range(4):
                ps = pps.tile([128, C + 1], F32, tag="preps257", bufs=2)
                for ct in range(2):
                    nc.tensor.matmul(
                        ps[:],
                        V_allT_sb[ct][:, jt * 128:(jt + 1) * 128],
                        OW_sb[ct][:],
                        start=(ct == 0), stop=False,
                    )
                nc.tensor.matmul(
                    ps[:], ones1_sb[:], ob_sb[:],
                    start=False, stop=True,
                )
                t = cp.tile([128, C + 1], BF16, tag=f"v2_{jt}")
                nc.vector.tensor_copy(t[:], ps[:])
                V2_sb.append(t)

        # ---- main per-pixel loop ----
        io = es.enter_context(tc.tile_pool(name="io", bufs=4))
        sps = es.enter_context(tc.tile_pool(name="sps", space="PSUM", bufs=5))
        ops = es.enter_context(tc.tile_pool(name="ops", space="PSUM", bufs=2))

        for mt in range(NMT):
            fT_t = []
            for et in range(2):
                t = io.tile([128, TP], BF16, tag=f"ft{et}", bufs=3)
                nc.sync.dma_start(
                    t[:], fT[et * 128:(et + 1) * 128, mt * TP:(mt + 1) * TP])
                fT_t.append(t)
            ET_t = io.tile([M, TP], BF16, tag="et", bufs=3)
            nc.sync.dma_start(ET_t[:], ET[:, mt * TP:(mt + 1) * TP])
            gate_t = io.tile([128, 4], F32, tag="gate", bufs=3)
            nc.sync.dma_start(gate_t[:], gate[mt, :, :])
            fpm_t = io.tile([128, 4 * C], F32, tag="fpm", bufs=3)
            nc.sync.dma_start(fpm_t[:], fpm[mt, :, :])

            P_t = []
            for jt in range(4):
                sp = sps.tile([128, TP], F32, tag="s", bufs=5)
                nc.tensor.matmul(
                    sp[:], KWT_sb[0][:, jt * 128:(jt + 1) * 128],
                    fT_t[0][:], start=True, stop=False)
                nc.tensor.matmul(
                    sp[:], KWT_sb[1][:, jt * 128:(jt + 1) * 128],
                    fT_t[1][:], start=False, stop=False)
                nc.tensor.matmul(
                    sp[:], R30_sb[:, jt * 128:(jt + 1) * 128],
                    ET_t[:], start=False, stop=True)
                pt = io.tile([128, TP], BF16, tag=f"p{jt}", bufs=2)
                nc.scalar.activation(pt[:], sp[:], AF.Exp, bias=sb_jm[jt][:, 0:1])
                P_t.append(pt)

            otb = io.tile([128, 4 * C], F32, tag="otb", bufs=3)
            for st in range(4):
                op = ops.tile([128, C + 1], F32, tag="o", bufs=2)
                for jt in range(4):
                    nc.tensor.matmul(
                        op[:], P_t[jt][:, st * 128:(st + 1) * 128],
                        V2_sb[jt][:], start=(jt == 0), stop=(jt == 3))
                recip = io.tile([128, 1], F32, tag="recip", bufs=4)
                nc.vector.reciprocal(recip[:], op[:, C:C + 1])
                rg = io.tile([128, 1], F32, tag="rg", bufs=4)
                nc.vector.tensor_mul(rg[:], recip[:], gate_t[:, st:st + 1])
                nc.vector.scalar_tensor_tensor(
                    otb[:, st * C:(st + 1) * C], op[:, 0:C], rg[:, 0:1],
                    fpm_t[:, st * C:(st + 1) * C], OP.mult, OP.add)
            nc.sync.dma_start(out[mt, :, :], otb[:])

    nc.compile()
    return nc


_CACHE = {}


def _build():
    if "nc" not in _CACHE:
        _CACHE["nc"] = build_nc()
    return _CACHE["nc"]


def _prep_maps(anchors, features, instances_in_view, in_proj_w, in_proj_b,
               out_w, out_b):
    f32 = np.float32
    bf16 = ml_dtypes.bfloat16
    anchors = np.asarray(anchors, f32)
    features = np.asarray(features, f32)
    iiv = np.asarray(instances_in_view, np.int32)
    in_proj_w = np.asarray(in_proj_w, f32)
    in_proj_b = np.asarray(in_proj_b, f32)
    out_w = np.asarray(out_w, f32)
    out_b = np.asarray(out_b, f32)

    f_flat = features.reshape(N_FULL, C)
    fT_full = np.ascontiguousarray(f_flat.T.astype(bf16))
    lab = iiv.reshape(-1)
    idx = np.maximum(lab - 1, 0)
    ET_full = (idx[None, :] == np.arange(M, dtype=np.int32)[:, None]).astype(bf16)
    gate_full = (lab > 0).astype(f32)

    anchorsT = np.ascontiguousarray(anchors.reshape(J, C).T)
    Wq_h = np.ascontiguousarray(in_proj_w[:C])
    WkT_h = np.ascontiguousarray(in_proj_w[C:2 * C].T)
    WvT_h = np.ascontiguousarray(in_proj_w[2 * C:].T)
    bq_h = np.ascontiguousarray(in_proj_b[:C].reshape(C, 1))
    bk_h = np.ascontiguousarray(in_proj_b[C:2 * C].reshape(C, 1))
    bv_h = np.ascontiguousarray(in_proj_b[2 * C:].reshape(C, 1))
    OWaug_h = np.concatenate([out_w.T, np.zeros((C, 1), f32)], axis=1)
    obaug_h = np.concatenate([out_b, np.ones(1, f32)]).reshape(1, C + 1)
    R30_h = (np.repeat(np.eye(M, dtype=f32), L, axis=1) * f32(BIG)).astype(bf16)
    ones1_h = np.ones((1, 128), bf16)

    in_maps = []
    for i in range(NCORES):
        sl = slice(i * NP, (i + 1) * NP)
        in_maps.append({
            "fT": np.ascontiguousarray(fT_full[:, sl]),
            "fpm": np.ascontiguousarray(
                f_flat[sl].reshape(NMT, 4, 128, C).transpose(0, 2, 1, 3)
                .reshape(NMT, 128, 4 * C)),
            "ET": np.ascontiguousarray(ET_full[:, sl]),
            "gate": np.ascontiguousarray(
                gate_full[sl].reshape(NMT, 4, 128).transpose(0, 2, 1)),
            "anchorsT": anchorsT, "Wq": Wq_h, "WkT": WkT_h, "WvT": WvT_h,
            "bq": bq_h, "bk": bk_h, "bv": bv_h,
            "OWaug": OWaug_h, "obaug": obaug_h,
            "R30": R30_h, "ones1": ones1_h,
        })
    return in_maps, features.shape


def _run(in_maps, **kw):
    nc = _build()
    return run_bass_kernel_spmd(nc, in_maps, core_ids=list(range(NCORES)), **kw)


def kernel(**inputs):
    in_maps, shp = _prep_maps(**inputs)
    res = _run(in_maps)
    outs = [
        np.asarray(r["out"]).reshape(NMT, 128, 4, C).transpose(0, 2, 1, 3)
        .reshape(NP, C)
        for r in res.results
    ]
    return np.concatenate(outs, axis=0).reshape(shp).astype(np.float32)
